# revision 17
# baseline (speedup 1.0000x reference)
"""DoubleStreamBlock (Flux-style) TRN2 Bass kernel, Megatron TP-8.

Sharding: 24 heads / qkv / mlp columns split 8 ways (3 heads, 1152 qkv cols,
1536 mlp cols per core); proj / mlp_w2 row-split with ReduceScatter; LN stats
via tiny AllReduce; h2 via AllGather.  All activations live feature-major
(channels on partitions, tokens on the free axis) so every matmul contracts
over the partition dim with no transposes (except small V-transposes on PE).

Token order inside the kernel is [img(2048) | txt(5)] (attention is
permutation-equivariant; pe is reordered to match on the host).
"""

import os
import sys
import contextlib

import numpy as np
import ml_dtypes

for _p in ("/opt/trn_rl_repo", os.path.expanduser("~/.axon_site/_ro/trn_rl_repo")):
    if os.path.isdir(_p) and _p not in sys.path:
        sys.path.insert(0, _p)

import concourse.bass as bass
import concourse.bacc as bacc
import concourse.tile as tile
from concourse import mybir
from concourse.masks import make_identity

BF = mybir.dt.bfloat16
F32 = mybir.dt.float32
AF = mybir.ActivationFunctionType
OP = mybir.AluOpType

NC = 8
B, HID, HEADS, HD, MLPD = 2, 3072, 24, 128, 12288
TXT, IMG_S = 5, 2048
L = IMG_S + TXT          # 2053, internal order [img | txt]
KC = HID // 128          # 24 hid chunks
HPC = HEADS // NC        # 3 heads per core
QKVC = 3 * HPC * HD      # 1152 per-core qkv cols (q|k|v each 384)
MLPC = MLPD // NC        # 1536
CH = HID // NC           # 384 channel shard
MC = MLPC // 128         # 12 mlp chunks per core
CHC = CH // 128          # 3 shard chunks
NT = 17                  # 128-token chunks along L (16 full + 5)
ISQ = float(1.0 / np.sqrt(HD))
EPS = 1e-6

# token tiles along L: (offset, size, stream) ; stream 0=img 1=txt
QT = [(0, 512, 0), (512, 512, 0), (1024, 512, 0), (1536, 512, 0), (2048, TXT, 1)]
# k chunks along L for attention
KCH = [(i * 128, 128) for i in range(16)] + [(2048, TXT)]

bf16 = ml_dtypes.bfloat16


def build_program():
    nc = bacc.Bacc("TRN2", target_bir_lowering=False, debug=False, num_devices=NC)

    ins = {}

    def inp(name, shape, dt):
        ins[name] = nc.dram_tensor(name, list(shape), dt, kind="ExternalInput").ap()
        return ins[name]

    xT = inp("xT", (B, HID, L), BF)                 # full tokens, all cores
    xT_sh = inp("xT_sh", (B, CH, L), BF)            # channel shard (stats+residual)
    vecT = inp("vecT", (HID, B), BF)
    inp("peA_q", (B, HD, L), BF)
    inp("peB_q", (B, HD, L), BF)
    inp("peA_k", (B, HD, L), BF)
    inp("peB_k", (B, HD, L), BF)
    wmod = inp("wmod", (HID, 4608), BF)
    inp("bmodT", (128, 36), F32)
    for s in ("i", "t"):
        inp(f"{s}_wqkv", (HID, QKVC), BF)
        inp(f"{s}_bqkvT", (128, 9), F32)
        inp(f"{s}_wproj", (CH, HID), BF)
        inp(f"{s}_bprojT", (128, CHC), F32)
        inp(f"{s}_w1", (HID, MLPC), BF)
        inp(f"{s}_b1T", (128, MC), F32)
        inp(f"{s}_w2", (MLPC, HID), BF)
        inp(f"{s}_b2T", (128, CHC), F32)

    outT_img = nc.dram_tensor("outT_img", [B, CH, IMG_S], F32,
                              kind="ExternalOutput").ap()
    outT_txt = nc.dram_tensor("outT_txt", [B, CH, TXT], F32,
                              kind="ExternalOutput").ap()

    GRP = [list(range(NC))]

    with tile.TileContext(nc) as tc:
        with contextlib.ExitStack() as ctx:
            dram = ctx.enter_context(tc.tile_pool(name="dram", bufs=1, space="DRAM"))
            const = ctx.enter_context(tc.tile_pool(name="const", bufs=1, side="right"))
            rows = ctx.enter_context(tc.tile_pool(name="rows", bufs=1, side="right"))
            psum = ctx.enter_context(tc.tile_pool(name="psum", bufs=1, space="PSUM"))

            # ---------------- internal DRAM ----------------
            ag_mod_in = dram.tile([128, 12, B], F32)
            ag_mod_out = dram.tile([NC, 128, 12, B], F32, addr_space="Shared")
            ar_s1_in = dram.tile([2, B, L], F32)
            ar_s1_out = dram.tile([2, B, L], F32, addr_space="Shared")
            rs1_in = dram.tile([HID, B, L], F32)
            rs1_out = dram.tile([CH, B, L], F32)
            ar_s2_in = dram.tile([2, B, L], F32)
            ar_s2_out = dram.tile([2, B, L], F32, addr_space="Shared")
            ag_h2_in = dram.tile([CH, B, L], BF)
            ag_h2_out = dram.tile([HID, B, L], BF, addr_space="Shared")
            rs2_in = dram.tile([HID, B, L], F32)
            rs2_out = dram.tile([CH, B, L], F32)
            xo_spill = dram.tile([B, CH, L], F32)
            v_dram = dram.tile([B, NT * 128, 3 * HD], BF)

            # ---------------- constants ----------------
            ident = const.tile([128, 128], BF)
            make_identity(nc, ident)
            ones_bf = const.tile([128, 1], BF)
            nc.vector.memset(ones_bf, 1.0)
            eps_sb = const.tile([128, 1], F32)
            nc.vector.memset(eps_sb, EPS)
            modres = const.tile([128, 36, B], F32)
            bias_sb = {}
            for nm, w in (("bmodT", 36), ("i_bqkvT", 9), ("t_bqkvT", 9),
                          ("i_bprojT", CHC), ("t_bprojT", CHC),
                          ("i_b1T", MC), ("t_b1T", MC),
                          ("i_b2T", CHC), ("t_b2T", CHC)):
                tl = const.tile([128, w], F32, name=f"bias_{nm}")
                nc.sync.dma_start(out=tl, in_=ins[nm])
                bias_sb[nm] = tl

            def ps_mm(tag="mm", shape=(128, 512), bufs=2, dt=F32):
                return psum.tile(list(shape), dt, tag=tag, bufs=bufs,
                                 name=f"ps_{tag}")

            # ================= P0: mod matmul + LN1 stats =================
            with tc.tile_pool(name="p0", bufs=3) as p0:
                sv = const.tile([128, KC, B], BF)
                vec_sb = p0.tile([128, KC, B], BF, tag="vec")
                nc.sync.dma_start(out=vec_sb,
                                  in_=vecT.rearrange("(k p) b -> p k b", p=128))
                nc.scalar.activation(out=sv, in_=vec_sb, func=AF.Silu)

                for t in range(36):
                    wcol = p0.tile([128, KC, 128], BF, tag="wcol")
                    nc.sync.dma_start(
                        out=wcol,
                        in_=wmod[:, t * 128:(t + 1) * 128].rearrange(
                            "(k p) n -> p k n", p=128))
                    ps = ps_mm("row", (128, 512))
                    for k in range(KC):
                        nc.tensor.matmul(ps[:, :B], wcol[:, k, :], sv[:, k, :],
                                         start=(k == 0), stop=(k == KC - 1))
                    nc.vector.tensor_scalar_add(
                        out=modres[:, t, :], in0=ps[:, :B],
                        scalar1=bias_sb["bmodT"][:, t:t + 1])

                # ship sh1/sc1 (cols 0..11 of modres) for allgather
                nc.sync.dma_start(out=ag_mod_in, in_=modres[:, 0:12, :])

                # LN1 stats partials from the channel shard (raw x)
                for b in range(B):
                    xsh = p0.tile([128, CHC, L], BF, tag="xsh", bufs=2)
                    nc.sync.dma_start(
                        out=xsh, in_=xT_sh[b].rearrange("(c p) t -> p c t", p=128))
                    x2 = p0.tile([128, CHC, L], BF, tag="x2", bufs=2)
                    nc.vector.tensor_mul(x2, xsh, xsh)
                    for (o, n, _s) in QT:
                        ps1 = ps_mm("cs", (1, 512), bufs=1)
                        ps2 = ps_mm("row", (128, 512))
                        for kk in range(CHC):
                            nc.tensor.matmul(ps1[:, :n], ones_bf,
                                             xsh[:, kk, o:o + n],
                                             start=(kk == 0), stop=(kk == CHC - 1))
                            nc.tensor.matmul(ps2[:1, :n], ones_bf,
                                             x2[:, kk, o:o + n],
                                             start=(kk == 0), stop=(kk == CHC - 1))
                        st1 = p0.tile([1, 512], F32, tag="st1", bufs=2)
                        st2 = p0.tile([1, 512], F32, tag="st2", bufs=2)
                        nc.vector.tensor_copy(out=st1[:, :n], in_=ps1[:, :n])
                        nc.vector.tensor_copy(out=st2[:, :n], in_=ps2[:1, :n])
                        nc.sync.dma_start(out=ar_s1_in[0, b, o:o + n],
                                          in_=st1[:, :n])
                        nc.sync.dma_start(out=ar_s1_in[1, b, o:o + n],
                                          in_=st2[:, :n])

                nc.gpsimd.collective_compute(
                    "AllGather", OP.bypass, replica_groups=GRP,
                    ins=[ag_mod_in.opt()], outs=[ag_mod_out.opt()])
                nc.gpsimd.collective_compute(
                    "AllReduce", OP.add, replica_groups=GRP,
                    ins=[ar_s1_in.opt()], outs=[ar_s1_out.opt()])

            # -------- P0post: full mod vectors + LN1 rows --------
            full_mod = {}
            opsc1 = {}
            opsc2 = {}
            uvrhs_bf = {}
            with tc.tile_pool(name="p0b", bufs=1) as p0b:
                for qi, qname in enumerate(("i_sh1", "i_sc1", "t_sh1", "t_sc1")):
                    tl = const.tile([128, KC, B], F32, name=f"fm_{qname}")
                    for j in range(3):
                        nc.sync.dma_start(
                            out=tl.rearrange("p (c j) b -> p j c b", j=3)[:, j],
                            in_=ag_mod_out.rearrange(
                                "c p (q j) b -> q j p c b", q=4)[qi, j])
                    full_mod[qname] = tl
                for s in ("i", "t"):
                    tl = const.tile([128, KC, B], F32, name=f"opsc1_{s}")
                    nc.vector.tensor_scalar_add(out=tl, in0=full_mod[f"{s}_sc1"],
                                                scalar1=1.0)
                    opsc1[s] = tl
                    si = 0 if s == "i" else 1
                    t2 = const.tile([128, CHC, B], F32, name=f"opsc2_{s}")
                    nc.vector.tensor_scalar_add(
                        out=t2,
                        in0=modres[:, 12 + si * 12 + 2 * CHC:12 + si * 12 + 3 * CHC, :],
                        scalar1=1.0)
                    opsc2[s] = t2
                    # uv rhs: [opsc1_b0, sh1_b0, opsc1_b1, sh1_b1]
                    uvf = p0b.tile([128, KC, 4], F32, tag="uvf")
                    for b in range(B):
                        nc.vector.tensor_copy(out=uvf[:, :, 2 * b],
                                              in_=opsc1[s][:, :, b])
                        nc.vector.tensor_copy(out=uvf[:, :, 2 * b + 1],
                                              in_=full_mod[f"{s}_sh1"][:, :, b])
                    tb = const.tile([128, KC, 4], BF, name=f"uvrhs_{s}")
                    nc.vector.tensor_copy(out=tb, in_=uvf)
                    uvrhs_bf[s] = tb

            # mod locals: modres col 12 + si*12 + q*3 + kk  (q: 0=g1 1=sh2 2=sc2 3=g2)
            def mloc(s, q, kk, b):
                si = 0 if s == "i" else 1
                return modres[:, 12 + si * 12 + q * 3 + kk, b:b + 1]

            def token_norm_rows(ar_out, mt, rt):
                """ar_out [2,B,L] sums in DRAM -> m rows mt[b], rstd rows rt[b]."""
                with tc.tile_pool(name="tnr", bufs=1) as tp:
                    sf = tp.tile([4, L], F32, tag="sf")
                    nc.sync.dma_start(out=sf,
                                      in_=ar_out.rearrange("k b l -> (k b) l"))
                    for b in range(B):
                        s1 = tp.tile([1, L], F32, tag="s1r")
                        s2 = tp.tile([1, L], F32, tag="s2r")
                        nc.sync.dma_start(out=s1, in_=sf[b:b + 1, :])
                        nc.sync.dma_start(out=s2, in_=sf[2 + b:3 + b, :])
                        nc.vector.tensor_scalar_mul(out=mt[b], in0=s1,
                                                    scalar1=1.0 / HID)
                        e2 = tp.tile([1, L], F32, tag="e2")
                        nc.vector.tensor_scalar_mul(out=e2, in0=s2,
                                                    scalar1=1.0 / HID)
                        msq = tp.tile([1, L], F32, tag="msq")
                        nc.vector.tensor_mul(msq, mt[b], mt[b])
                        var = tp.tile([1, L], F32, tag="var")
                        nc.vector.tensor_sub(var, e2, msq)
                        sd = tp.tile([1, L], F32, tag="sd")
                        nc.scalar.activation(out=sd, in_=var, func=AF.Sqrt,
                                             bias=eps_sb[0:1, :])
                        nc.vector.reciprocal(out=rt[b], in_=sd)

            m1_r = [rows.tile([1, L], F32, name=f"m1_{b}", tag=f"m_{b}")
                    for b in range(B)]
            r1_r = [rows.tile([1, L], F32, name=f"r1_{b}", tag=f"r_{b}")
                    for b in range(B)]
            token_norm_rows(ar_s1_out, m1_r, r1_r)

            # ================= P1: u/v vectors =================
            uv_sb = {}
            with tc.tile_pool(name="p1uv", bufs=3) as p1uv:
                for s in ("i", "t"):
                    w = ins[f"{s}_wqkv"]
                    uv = const.tile([128, 9, 4], F32, name=f"uv_{s}")
                    for t in range(9):
                        wcol = p1uv.tile([128, KC, 128], BF, tag="uvw", bufs=2)
                        nc.sync.dma_start(
                            out=wcol,
                            in_=w[:, t * 128:(t + 1) * 128].rearrange(
                                "(k p) n -> p k n", p=128))
                        ps = ps_mm("row", (128, 512))
                        for k in range(KC):
                            nc.tensor.matmul(ps[:, :4], wcol[:, k, :],
                                             uvrhs_bf[s][:, k, :],
                                             start=(k == 0), stop=(k == KC - 1))
                        for b in range(B):
                            nc.vector.tensor_scalar_mul(
                                out=uv[:, t, 2 * b:2 * b + 1],
                                in0=ps[:, 2 * b:2 * b + 1], scalar1=-1.0)
                            nc.vector.tensor_scalar_add(
                                out=uv[:, t, 2 * b + 1:2 * b + 2],
                                in0=ps[:, 2 * b + 1:2 * b + 2],
                                scalar1=bias_sb[f"{s}_bqkvT"][:, t:t + 1])
                    uv_sb[s] = uv

            # ================= P1: qkv matmuls =================
            at_pool = tc.alloc_tile_pool(name="at", bufs=1, side="right")
            attn_sb = {b: at_pool.tile([128, HPC, L], BF, name=f"attn_{b}")
                       for b in range(B)}
            qk_pool = tc.alloc_tile_pool(name="qk", bufs=1, side="right")
            q_sb, k_sb = {}, {}
            for b in range(B):
                for h in range(HPC):
                    q_sb[(b, h)] = qk_pool.tile([128, L], BF, name=f"q_{b}_{h}")
                    k_sb[(b, h)] = qk_pool.tile([128, L], BF, name=f"k_{b}_{h}")

            with tc.tile_pool(name="p1", bufs=1) as p1:
                wq_sb = p1.tile([128, KC, QKVC], BF, tag="wq")
                nc.sync.dma_start(
                    out=wq_sb,
                    in_=ins["i_wqkv"].rearrange("(k p) n -> p k n", p=128))
                for b in range(B):
                    # txt x, scaled by (1+sc1_txt)
                    xs_txt = p1.tile([128, KC, TXT], BF, tag="xstxt", bufs=2)
                    xt_raw = p1.tile([128, KC, TXT], BF, tag="xtraw", bufs=2)
                    nc.sync.dma_start(
                        out=xt_raw,
                        in_=xT[b, :, IMG_S:].rearrange("(k p) t -> p k t", p=128))
                    for k in range(KC):
                        nc.vector.tensor_scalar_mul(
                            out=xs_txt[:, k, :], in0=xt_raw[:, k, :],
                            scalar1=opsc1["t"][:, k, b:b + 1])

                    for (o, n, si) in QT:
                        s = "it"[si]
                        if si == 0:
                            xtile = p1.tile([128, KC, 512], BF, tag="xtile")
                            nc.sync.dma_start(
                                out=xtile[:, :, :n],
                                in_=xT[b, :, o:o + n].rearrange(
                                    "(k p) t -> p k t", p=128))
                            for k in range(KC):   # x *= (1+sc1) in place
                                nc.vector.tensor_scalar_mul(
                                    out=xtile[:, k, :n], in0=xtile[:, k, :n],
                                    scalar1=opsc1["i"][:, k, b:b + 1])
                        else:
                            xtile = xs_txt
                        m_bc = p1.tile([128, 512], F32, tag="mbc")
                        r_bc = p1.tile([128, 512], F32, tag="rbc")
                        nc.gpsimd.partition_broadcast(m_bc[:, :n],
                                                      m1_r[b][:, o:o + n])
                        nc.gpsimd.partition_broadcast(r_bc[:, :n],
                                                      r1_r[b][:, o:o + n])
                        for ct in range(9):
                            ps = ps_mm("mm")
                            wtxt = None
                            for k in range(KC):
                                if si == 0:
                                    lhsT = wq_sb[:, k, ct * 128:(ct + 1) * 128]
                                else:
                                    if k == 0:
                                        wtxt = p1.tile([128, KC, 128], BF,
                                                       tag="wtxt")
                                        nc.sync.dma_start(
                                            out=wtxt,
                                            in_=ins["t_wqkv"]
                                            [:, ct * 128:(ct + 1) * 128]
                                            .rearrange("(k p) n -> p k n", p=128))
                                    lhsT = wtxt[:, k, :]
                                nc.tensor.matmul(ps[:, :n], lhsT, xtile[:, k, :n],
                                                 start=(k == 0), stop=(k == KC - 1))
                            # epilogue: (A - m*u) * rstd + v0
                            t1 = p1.tile([128, 512], F32, tag="ep1")
                            nc.vector.scalar_tensor_tensor(
                                out=t1[:, :n], in0=m_bc[:, :n],
                                scalar=uv_sb[s][:, ct, 2 * b:2 * b + 1],
                                in1=ps[:, :n], op0=OP.mult, op1=OP.add)
                            t2 = p1.tile([128, 512], F32, tag="ep2")
                            nc.vector.tensor_mul(t2[:, :n], t1[:, :n], r_bc[:, :n])
                            if ct < 6:
                                dest = (q_sb[(b, ct)] if ct < HPC
                                        else k_sb[(b, ct - HPC)])
                                nc.vector.tensor_scalar_add(
                                    out=dest[:, o:o + n], in0=t2[:, :n],
                                    scalar1=uv_sb[s][:, ct, 2 * b + 1:2 * b + 2])
                            else:
                                vst = p1.tile([128, 512], BF, tag="vst", bufs=2)
                                nc.vector.tensor_scalar_add(
                                    out=vst[:, :n], in0=t2[:, :n],
                                    scalar1=uv_sb[s][:, ct, 2 * b + 1:2 * b + 2])
                                h = ct - 6
                                for i in range((n + 127) // 128):
                                    nn_ = min(128, n - i * 128)
                                    tps = ps_mm("sc", (128, 512), dt=BF)
                                    nc.tensor.transpose(
                                        tps[:nn_, :128],
                                        vst[:, i * 128:i * 128 + nn_], ident)
                                    vt = p1.tile([128, 128], BF, tag="vt", bufs=2)
                                    nc.vector.tensor_copy(out=vt[:nn_, :],
                                                          in_=tps[:nn_, :128])
                                    nc.sync.dma_start(
                                        out=v_dram[b, o + i * 128:o + i * 128 + nn_,
                                                   h * HD:(h + 1) * HD],
                                        in_=vt[:nn_, :])

            # ================= P1b: qknorm + rope =================
            with tc.tile_pool(name="p1r", bufs=1) as p1r:
                for b in range(B):
                    pe_sb = {}
                    for nm in ("peA_q", "peB_q", "peA_k", "peB_k"):
                        tl = p1r.tile([128, L], BF, tag=nm)
                        nc.sync.dma_start(out=tl, in_=ins[nm][b])
                        pe_sb[nm] = tl
                    for h in range(HPC):
                        for sd_i, (t_sb, pa, pb) in enumerate(
                                ((q_sb[(b, h)], "peA_q", "peB_q"),
                                 (k_sb[(b, h)], "peA_k", "peB_k"))):
                            sq = p1r.tile([128, L], BF, tag="sq")
                            nc.vector.tensor_mul(sq, t_sb, t_sb)
                            rr = p1r.tile([1, L], F32, tag="rr")
                            for (o, n, _s) in QT:
                                psr = ps_mm("cs", (1, 512), bufs=1)
                                nc.tensor.matmul(psr[:, :n], ones_bf,
                                                 sq[:, o:o + n],
                                                 start=True, stop=True)
                                nc.vector.tensor_copy(out=rr[:, o:o + n],
                                                      in_=psr[:, :n])
                            sdt = p1r.tile([1, L], F32, tag="rsd")
                            nc.scalar.activation(out=sdt, in_=rr, func=AF.Sqrt,
                                                 scale=1.0 / HD,
                                                 bias=eps_sb[0:1, :])
                            rinv = p1r.tile([1, L], F32, tag="rinv")
                            nc.vector.reciprocal(out=rinv, in_=sdt)
                            rb = p1r.tile([128, L], F32, tag="rmsbc")
                            nc.gpsimd.partition_broadcast(rb, rinv)
                            qn = p1r.tile([128, L], BF, tag="qn")
                            nc.vector.tensor_mul(qn, t_sb, rb)
                            qsw = p1r.tile([128, L], BF, tag="qsw")
                            nc.sync.dma_start(out=qsw[0:64, :], in_=qn[64:128, :])
                            nc.sync.dma_start(out=qsw[64:128, :], in_=qn[0:64, :])
                            t1 = p1r.tile([128, L], BF, tag="rp1")
                            nc.vector.tensor_mul(t1, pe_sb[pa], qn)
                            t2 = p1r.tile([128, L], BF, tag="rp2")
                            nc.vector.tensor_mul(t2, pe_sb[pb], qsw)
                            nc.vector.tensor_add(t_sb, t1, t2)

            # ================= P2: attention =================
            with tc.tile_pool(name="p2", bufs=1) as p2:
                for b in range(B):
                    for h in range(HPC):
                        vh = p2.tile([128, NT, HD], BF, tag="vh", bufs=2)
                        nc.sync.dma_start(
                            out=vh,
                            in_=v_dram[b, :, h * HD:(h + 1) * HD].rearrange(
                                "(c p) d -> p c d", p=128))
                        for (o, n, _s) in QT:
                            ps_av = ps_mm("av", bufs=1)
                            ps_cs = ps_mm("cs", (1, 512), bufs=1)
                            for ki, (ko, kn) in enumerate(KCH):
                                ps_s = ps_mm("sc")
                                nc.tensor.matmul(ps_s[:kn, :n],
                                                 k_sb[(b, h)][:, ko:ko + kn],
                                                 q_sb[(b, h)][:, o:o + n],
                                                 start=True, stop=True)
                                ex = p2.tile([128, 512], BF, tag="ex", bufs=3)
                                nc.scalar.activation(out=ex[:kn, :n],
                                                     in_=ps_s[:kn, :n],
                                                     func=AF.Exp, scale=ISQ)
                                nc.tensor.matmul(ps_av[:, :n],
                                                 vh[:kn, ko // 128, :],
                                                 ex[:kn, :n],
                                                 start=(ki == 0), stop=(ki == 16))
                                nc.tensor.matmul(ps_cs[:, :n],
                                                 ones_bf[:kn], ex[:kn, :n],
                                                 start=(ki == 0), stop=(ki == 16))
                            csr = p2.tile([1, 512], F32, tag="csr", bufs=2)
                            nc.vector.reciprocal(out=csr[:, :n], in_=ps_cs[:, :n])
                            cbc = p2.tile([128, 512], F32, tag="cbc", bufs=2)
                            nc.gpsimd.partition_broadcast(cbc[:, :n], csr[:, :n])
                            nc.vector.tensor_mul(attn_sb[b][:, h, o:o + n],
                                                 ps_av[:, :n], cbc[:, :n])

            qk_pool.release()

            # ================= P3: proj partial + RS1 =================
            with tc.tile_pool(name="p3", bufs=1) as p3:
                wproj_sb = {}
                for s in ("i", "t"):
                    tl = p3.tile([128, CHC, HID], BF, tag=f"wproj_{s}")
                    nc.sync.dma_start(
                        out=tl,
                        in_=ins[f"{s}_wproj"].rearrange("(c p) n -> p c n", p=128))
                    wproj_sb[s] = tl
                for b in range(B):
                    for (o, n, si) in QT:
                        s = "it"[si]
                        for ct in range(KC):
                            ps = ps_mm("mm")
                            for hc in range(CHC):
                                nc.tensor.matmul(
                                    ps[:, :n],
                                    wproj_sb[s][:, hc, ct * 128:(ct + 1) * 128],
                                    attn_sb[b][:, hc, o:o + n],
                                    start=(hc == 0), stop=(hc == CHC - 1))
                            st = p3.tile([128, 512], F32, tag="prst", bufs=3)
                            nc.vector.tensor_copy(out=st[:, :n], in_=ps[:, :n])
                            nc.sync.dma_start(
                                out=rs1_in[ct * 128:(ct + 1) * 128, b, o:o + n],
                                in_=st[:, :n])
                nc.gpsimd.collective_compute(
                    "ReduceScatter", OP.add, replica_groups=GRP,
                    ins=[rs1_in.opt()], outs=[rs1_out.opt()])

            at_pool.release()

            # ================= P4: residual + LN2 stats =================
            xo_pool = tc.alloc_tile_pool(name="xop", bufs=1, side="right")
            xo_bf = {}
            with tc.tile_pool(name="p4", bufs=1) as p4:
                for b in range(B):
                    x2l = []
                    for kk in range(CHC):
                        rt = p4.tile([128, L], F32, tag="rt", bufs=2)
                        nc.sync.dma_start(
                            out=rt, in_=rs1_out[kk * 128:(kk + 1) * 128, b, :])
                        res = p4.tile([128, L], BF, tag="res", bufs=2)
                        nc.sync.dma_start(
                            out=res, in_=xT_sh[b, kk * 128:(kk + 1) * 128, :])
                        resf = p4.tile([128, L], F32, tag="resf")
                        nc.vector.tensor_copy(out=resf, in_=res)
                        xo = p4.tile([128, L], F32, tag="xo")
                        for s, o, n in (("i", 0, IMG_S), ("t", IMG_S, TXT)):
                            t1 = p4.tile([128, L], F32, tag="p4t1")
                            nc.vector.tensor_scalar(
                                out=t1[:, o:o + n], in0=rt[:, o:o + n],
                                scalar1=bias_sb[f"{s}_bprojT"][:, kk:kk + 1],
                                scalar2=mloc(s, 0, kk, b),
                                op0=OP.add, op1=OP.mult)
                            nc.vector.tensor_add(xo[:, o:o + n], t1[:, o:o + n],
                                                 resf[:, o:o + n])
                        nc.sync.dma_start(
                            out=xo_spill[b, kk * 128:(kk + 1) * 128, :], in_=xo)
                        xb = xo_pool.tile([128, L], BF, tag=f"xob{b}{kk}",
                                          name=f"xob_{b}_{kk}")
                        nc.vector.tensor_copy(out=xb, in_=xo)
                        xo_bf[(b, kk)] = xb
                        x2 = p4.tile([128, L], BF, tag="xo2", bufs=3)
                        nc.vector.tensor_mul(x2, xb, xb)
                        x2l.append(x2)
                    for (o, n, _s) in QT:
                        ps1 = ps_mm("cs", (1, 512), bufs=1)
                        ps2 = ps_mm("row", (128, 512))
                        for kk in range(CHC):
                            nc.tensor.matmul(ps1[:, :n], ones_bf,
                                             xo_bf[(b, kk)][:, o:o + n],
                                             start=(kk == 0), stop=(kk == CHC - 1))
                            nc.tensor.matmul(ps2[:1, :n], ones_bf,
                                             x2l[kk][:, o:o + n],
                                             start=(kk == 0), stop=(kk == CHC - 1))
                        st1 = p4.tile([1, 512], F32, tag="st1", bufs=2)
                        st2 = p4.tile([1, 512], F32, tag="st2", bufs=2)
                        nc.vector.tensor_copy(out=st1[:, :n], in_=ps1[:, :n])
                        nc.vector.tensor_copy(out=st2[:, :n], in_=ps2[:1, :n])
                        nc.sync.dma_start(out=ar_s2_in[0, b, o:o + n],
                                          in_=st1[:, :n])
                        nc.sync.dma_start(out=ar_s2_in[1, b, o:o + n],
                                          in_=st2[:, :n])
                nc.gpsimd.collective_compute(
                    "AllReduce", OP.add, replica_groups=GRP,
                    ins=[ar_s2_in.opt()], outs=[ar_s2_out.opt()])

            m2_r = [rows.tile([1, L], F32, name=f"m2_{b}", tag=f"m_{b}")
                    for b in range(B)]
            r2_r = [rows.tile([1, L], F32, name=f"r2_{b}", tag=f"r_{b}")
                    for b in range(B)]
            token_norm_rows(ar_s2_out, m2_r, r2_r)

            # ================= P5: h2 shard + AG =================
            with tc.tile_pool(name="p5", bufs=1) as p5:
                for b in range(B):
                    m2b = p5.tile([128, L], F32, tag="m2b")
                    r2b = p5.tile([128, L], F32, tag="r2b")
                    nc.gpsimd.partition_broadcast(m2b, m2_r[b])
                    nc.gpsimd.partition_broadcast(r2b, r2_r[b])
                    for kk in range(CHC):
                        t1 = p5.tile([128, L], F32, tag="p5t1", bufs=2)
                        nc.vector.tensor_sub(t1, xo_bf[(b, kk)], m2b)
                        t2 = p5.tile([128, L], F32, tag="p5t2", bufs=2)
                        nc.vector.tensor_mul(t2, t1, r2b)
                        h2 = p5.tile([128, L], BF, tag="h2", bufs=2)
                        for s, o, n in (("i", 0, IMG_S), ("t", IMG_S, TXT)):
                            nc.vector.tensor_scalar(
                                out=h2[:, o:o + n], in0=t2[:, o:o + n],
                                scalar1=opsc2[s][:, kk, b:b + 1],
                                scalar2=mloc(s, 1, kk, b),
                                op0=OP.mult, op1=OP.add)
                        nc.sync.dma_start(
                            out=ag_h2_in[kk * 128:(kk + 1) * 128, b, :], in_=h2)
                nc.gpsimd.collective_compute(
                    "AllGather", OP.bypass, replica_groups=GRP,
                    ins=[ag_h2_in.opt()], outs=[ag_h2_out.opt()])

            xo_pool.release()

            # ================= P6: MLP =================
            with tc.tile_pool(name="p6", bufs=1) as p6:
                w1_sb = p6.tile([128, KC, MLPC], BF, tag="w1sb")
                nc.sync.dma_start(
                    out=w1_sb, in_=ins["i_w1"].rearrange("(k p) n -> p k n", p=128))
                for b in range(B):
                    for (o, n, si) in QT:
                        s = "it"[si]
                        h2t = p6.tile([128, KC, 512], BF, tag="h2t")
                        nc.sync.dma_start(
                            out=h2t[:, :, :n],
                            in_=ag_h2_out[:, b, o:o + n].rearrange(
                                "(k p) t -> p k t", p=128))
                        gl = p6.tile([128, MC, 512], BF, tag="gl")
                        for mt in range(MC):
                            ps = ps_mm("mm")
                            w1t = None
                            for k in range(KC):
                                if si == 0:
                                    lhsT = w1_sb[:, k, mt * 128:(mt + 1) * 128]
                                else:
                                    if k == 0:
                                        w1t = p6.tile([128, KC, 128], BF,
                                                      tag="w1t", bufs=2)
                                        nc.sync.dma_start(
                                            out=w1t,
                                            in_=ins["t_w1"]
                                            [:, mt * 128:(mt + 1) * 128]
                                            .rearrange("(k p) n -> p k n", p=128))
                                    lhsT = w1t[:, k, :]
                                nc.tensor.matmul(ps[:, :n], lhsT, h2t[:, k, :n],
                                                 start=(k == 0), stop=(k == KC - 1))
                            nc.scalar.activation(
                                out=gl[:, mt, :n], in_=ps[:, :n],
                                func=AF.Gelu_apprx_tanh,
                                bias=bias_sb[f"{s}_b1T"][:, mt:mt + 1])
                        for ct in range(KC):
                            w2t = p6.tile([128, MC, 128], BF, tag="w2t", bufs=2)
                            nc.sync.dma_start(
                                out=w2t,
                                in_=ins[f"{s}_w2"][:, ct * 128:(ct + 1) * 128]
                                .rearrange("(k p) n -> p k n", p=128))
                            ps = ps_mm("mm")
                            for k in range(MC):
                                nc.tensor.matmul(ps[:, :n], w2t[:, k, :],
                                                 gl[:, k, :n],
                                                 start=(k == 0), stop=(k == MC - 1))
                            st = p6.tile([128, 512], F32, tag="w2st", bufs=3)
                            nc.vector.tensor_copy(out=st[:, :n], in_=ps[:, :n])
                            nc.sync.dma_start(
                                out=rs2_in[ct * 128:(ct + 1) * 128, b, o:o + n],
                                in_=st[:, :n])
                nc.gpsimd.collective_compute(
                    "ReduceScatter", OP.add, replica_groups=GRP,
                    ins=[rs2_in.opt()], outs=[rs2_out.opt()])

            # ================= P7: final epilogue =================
            with tc.tile_pool(name="p7", bufs=2) as p7:
                for b in range(B):
                    for kk in range(CHC):
                        rt = p7.tile([128, L], F32, tag="f_rt")
                        nc.sync.dma_start(
                            out=rt, in_=rs2_out[kk * 128:(kk + 1) * 128, b, :])
                        xo = p7.tile([128, L], F32, tag="f_xo")
                        nc.sync.dma_start(
                            out=xo, in_=xo_spill[b, kk * 128:(kk + 1) * 128, :])
                        ot = p7.tile([128, L], F32, tag="f_ot")
                        for s, o, n in (("i", 0, IMG_S), ("t", IMG_S, TXT)):
                            t1 = p7.tile([128, L], F32, tag="f_t1")
                            nc.vector.tensor_scalar(
                                out=t1[:, o:o + n], in0=rt[:, o:o + n],
                                scalar1=bias_sb[f"{s}_b2T"][:, kk:kk + 1],
                                scalar2=mloc(s, 3, kk, b),
                                op0=OP.add, op1=OP.mult)
                            nc.vector.tensor_add(ot[:, o:o + n], t1[:, o:o + n],
                                                 xo[:, o:o + n])
                        nc.sync.dma_start(
                            out=outT_img[b, kk * 128:(kk + 1) * 128, :],
                            in_=ot[:, :IMG_S])
                        nc.sync.dma_start(
                            out=outT_txt[b, kk * 128:(kk + 1) * 128, :],
                            in_=ot[:, IMG_S:])

    nc.compile()
    return nc


# --------------------------------------------------------------------------
# host-side prep
# --------------------------------------------------------------------------

_PERM = np.concatenate([np.arange(0, HD, 2), np.arange(1, HD, 2)])  # even|odd


def _prep_core_inputs(inp):
    """Build the 8 per-core in_maps (all numpy, bf16 where declared)."""
    img_s = inp["img"].reshape(B, 64, 64, HID)[:, :, :32, :].reshape(B, IMG_S, HID)
    x = np.concatenate([img_s, inp["txt"]], axis=1)          # [B, L, HID]
    xT = np.ascontiguousarray(x.transpose(0, 2, 1)).astype(bf16)

    # pe, internal order [img | txt]
    txt_pe = inp["pe"][:, :, :TXT]
    img_pe = inp["pe"][:, :, TXT:].reshape(B, 1, 64, 64, HD // 2, 2, 2)[:, :, :, :32]
    pe_s = np.concatenate(
        [img_pe.reshape(B, 1, IMG_S, HD // 2, 2, 2), txt_pe], axis=2)[:, 0]
    # pe_s [B, L, 64, 2, 2];  PA rows: top pe[.,0,0], bottom pe[.,1,1]
    #                         PB rows: top pe[.,0,1], bottom pe[.,1,0]
    pa = np.concatenate([pe_s[..., 0, 0], pe_s[..., 1, 1]], axis=-1)  # [B,L,128]
    pb = np.concatenate([pe_s[..., 0, 1], pe_s[..., 1, 0]], axis=-1)
    pa = pa.transpose(0, 2, 1)                                # [B,128,L]
    pb = pb.transpose(0, 2, 1)

    def scaled(base, scale_img, scale_txt):
        out = base.copy()
        out[:, :, :IMG_S] *= scale_img[_PERM][None, :, None]
        out[:, :, IMG_S:] *= scale_txt[_PERM][None, :, None]
        return np.ascontiguousarray(out).astype(bf16)

    common = {
        "xT": xT,
        "vecT": np.ascontiguousarray(inp["vec"].T).astype(bf16),
        "peA_q": scaled(pa, inp["img_q_scale"], inp["txt_q_scale"]),
        "peB_q": scaled(pb, inp["img_q_scale"], inp["txt_q_scale"]),
        "peA_k": scaled(pa, inp["img_k_scale"], inp["txt_k_scale"]),
        "peB_k": scaled(pb, inp["img_k_scale"], inp["txt_k_scale"]),
    }

    maps = []
    for c in range(NC):
        m = dict(common)
        m["xT_sh"] = np.ascontiguousarray(xT[:, c * CH:(c + 1) * CH, :])
        # mod cols: [i_sh1_sl, i_sc1_sl, t_sh1_sl, t_sc1_sl,
        #            i_g1, i_sh2, i_sc2, i_g2, t_g1, t_sh2, t_sc2, t_g2]
        cols, bcols = [], []
        for s in ("img", "txt"):
            w6 = inp[f"{s}_mod_w"].reshape(HID, 6, HID)
            b6 = inp[f"{s}_mod_b"].reshape(6, HID)
            cols += [w6[:, 0, c * CH:(c + 1) * CH], w6[:, 1, c * CH:(c + 1) * CH]]
            bcols += [b6[0, c * CH:(c + 1) * CH], b6[1, c * CH:(c + 1) * CH]]
        for s in ("img", "txt"):
            w6 = inp[f"{s}_mod_w"].reshape(HID, 6, HID)
            b6 = inp[f"{s}_mod_b"].reshape(6, HID)
            for q in (2, 3, 4, 5):   # g1, sh2, sc2, g2 shard slices
                cols.append(w6[:, q, c * CH:(c + 1) * CH])
                bcols.append(b6[q, c * CH:(c + 1) * CH])
        m["wmod"] = np.ascontiguousarray(np.concatenate(cols, axis=1)).astype(bf16)
        bm = np.concatenate(bcols)
        m["bmodT"] = np.ascontiguousarray(
            bm.reshape(36, 128).T).astype(np.float32)

        for s, sk in (("i", "img"), ("t", "txt")):
            w4 = inp[f"{sk}_qkv_w"].reshape(HID, 3, HEADS, HD)
            b4 = inp[f"{sk}_qkv_b"].reshape(3, HEADS, HD)
            blocks, bb = [], []
            for which in range(3):
                sl = w4[:, which, c * HPC:(c + 1) * HPC, :]
                bsl = b4[which, c * HPC:(c + 1) * HPC, :]
                if which < 2:           # q, k: permute head-dim
                    sl = sl[..., _PERM]
                    bsl = bsl[..., _PERM]
                blocks.append(sl.reshape(HID, HPC * HD))
                bb.append(bsl.reshape(HPC * HD))
            m[f"{s}_wqkv"] = np.ascontiguousarray(
                np.concatenate(blocks, axis=1)).astype(bf16)
            bq = np.concatenate(bb)
            m[f"{s}_bqkvT"] = np.ascontiguousarray(
                bq.reshape(9, 128).T).astype(np.float32)

            m[f"{s}_wproj"] = np.ascontiguousarray(
                inp[f"{sk}_proj_w"][c * CH:(c + 1) * CH, :]).astype(bf16)
            m[f"{s}_bprojT"] = np.ascontiguousarray(
                inp[f"{sk}_proj_b"].reshape(KC, 128).T[:, c * CHC:(c + 1) * CHC]
            ).astype(np.float32)
            m[f"{s}_w1"] = np.ascontiguousarray(
                inp[f"{sk}_mlp_w1"][:, c * MLPC:(c + 1) * MLPC]).astype(bf16)
            m[f"{s}_b1T"] = np.ascontiguousarray(
                inp[f"{sk}_mlp_b1"][c * MLPC:(c + 1) * MLPC].reshape(MC, 128).T
            ).astype(np.float32)
            m[f"{s}_w2"] = np.ascontiguousarray(
                inp[f"{sk}_mlp_w2"][c * MLPC:(c + 1) * MLPC, :]).astype(bf16)
            m[f"{s}_b2T"] = np.ascontiguousarray(
                inp[f"{sk}_mlp_b2"].reshape(KC, 128).T[:, c * CHC:(c + 1) * CHC]
            ).astype(np.float32)
        maps.append(m)
    return maps


def _assemble(results):
    img = np.concatenate([r["outT_img"] for r in results], axis=1)
    txt = np.concatenate([r["outT_txt"] for r in results], axis=1)
    img_o = np.ascontiguousarray(img.transpose(0, 2, 1))
    txt_o = np.ascontiguousarray(txt.transpose(0, 2, 1))
    return img_o, txt_o


_NC_CACHE = None


def _get_program():
    global _NC_CACHE
    if _NC_CACHE is None:
        _NC_CACHE = build_program()
    return _NC_CACHE


def kernel(**inputs):
    inp = {k: np.asarray(v) for k, v in inputs.items()}
    maps = _prep_core_inputs(inp)
    nc = _get_program()
    from concourse.bass_utils import run_bass_kernel_spmd
    res = run_bass_kernel_spmd(
        nc, maps, core_ids=list(range(NC)),
        trace=bool(int(os.environ.get("KERNEL_TRACE", "0"))))
    if res.exec_time_ns:
        print(f"HW exec time: {res.exec_time_ns} ns")
    return _assemble(res.results)


if __name__ == "__main__":
    nc = build_program()
    n_inst = sum(len(bb.instructions) for bb in nc.main_func.blocks)
    print("built OK, instructions:", n_inst)
